# revision 7
# baseline (speedup 1.0000x reference)
"""Trainium2 Bass kernel for nn_BasicNCAModel (neural cellular automaton).

Sharding: data-parallel over batch B=8 across 8 NeuronCores (1 image/core).
kernel() takes full inputs, shards per image on the host, runs the SPMD Bass
module via run_bass_kernel_spmd (PJRT under axon), and reassembles.

Per-core design (hardcoded for B=8, H=W=128, C=24, hidden=128, steps=8):
  - x lives ONLY as an fp16 master (ping-ponged per step), channel-major
    with a halo: partition 32g+c holds channel c of image rows
    [32g-1, 32g+32] (4 row-groups, 34 rows x 132 pitch), so circular
    padding becomes plain address offsets. fp16 rounding of the residual
    accumulates ~1e-3 rel over 8 steps (tolerance 2e-2).
  - perceive + W1 fuse into per-tap matrices A_t[k,c] = W1[k,24+c]*w1[t,c]
    + W1[k,48+c]*w2[t,c] (+W1[k,c] at the center tap). Per group a fp16
    "dx-stack" holds rows 24d+c = x16 shifted by dx=d-1 (shift baked into
    a contiguous DMA copy), row 72 = fire = (u<0.5) (host-precomputed),
    rows 73..127 = zero so K=128 (full-K matmuls run ~2x faster than
    partial-K). mm1 is 3 matmuls per 512-pixel tile (dy in {-1,0,1} via
    +-PITCH in the rhs AP), g-major so each group's PSUM bank stops early;
    the center one adds fire*M (M=512). Evacuation h'=relu(h+b1-M) is
    split ScalarE (cols 0:352, relu+bias) / VectorE (cols 352:512, STT
    add+max): masked pixels get relu(h-M)=0 so dx=0 exactly (fire trick).
  - mm2 (fp16, 4x col-tiled via tile_position (0,32g)) issues immediately
    after the group's evacuation: group g's dx lands at PSUM partitions
    32g..32g+31 (W2^T with channels 0..3 zeroed for the channel mask),
    so x16' = x16 + dx is one full-width DVE add per tile (f16 out). Two
    tiny adds write the circular wrap cols 0/129 straight from dx+x (no
    separate halo copies). The last step adds into an f32 staging tile
    that DMAs to the output. Stack rebuild DMAs are sliced and issued as
    soon as the adds they need complete (j-order [2,3,1,4,0,5,6,7] keeps
    cross-step dependencies off the tensor-engine critical path); stack
    halo rows come from the neighbor group's stack (DVE copies, top half
    mid-step, bottom half at step end).
"""

import sys

if "/opt/trn_rl_repo" not in sys.path:
    sys.path.insert(0, "/opt/trn_rl_repo")

import numpy as np

C = 24
NIC = 4
H = 128
WID = 128
HID = 128
STEPS = 8
B = 8
G = 4          # row groups
RG = 32        # image rows per group
PITCH = 132    # free-dim row pitch (130 used + 2 pad)
GROWS = 34     # rows incl halo
FB = GROWS * PITCH
TW = 512       # pixel tile = 4 image rows * 128 cols
JT = RG // 4   # tiles per group per step
M_FIRE = 512.0
SC_FD = 352    # ScalarE share of h' evacuation (cols 0:SC_FD)

_CACHE = {}


def _build_module():
    from concourse import bacc, mybir, tile

    f32 = mybir.dt.float32
    f16 = mybir.dt.float16
    Alu = mybir.AluOpType
    Act = mybir.ActivationFunctionType

    nc = bacc.Bacc(
        "TRN2",
        target_bir_lowering=False,
        debug=False,
        enable_asserts=False,
        num_devices=8,
    )

    apack = nc.dram_tensor("apack", [128, 384], f16, kind="ExternalInput").ap()
    w2p = nc.dram_tensor("w2p", [128, 32], f16, kind="ExternalInput").ap()
    b1col = nc.dram_tensor("b1col", [128, 1], f32, kind="ExternalInput").ap()
    x16in = nc.dram_tensor("x16in", [128, FB], f16, kind="ExternalInput").ap()
    firein = nc.dram_tensor("firein", [128, 4096], f16, kind="ExternalInput").ap()
    stkin = nc.dram_tensor("stkin", [128, 4 * FB], f16, kind="ExternalInput").ap()
    xout = nc.dram_tensor("xout", [128, 4096], f32, kind="ExternalOutput").ap()

    with tile.TileContext(nc) as tc:
        import contextlib

        with contextlib.ExitStack() as ctx:
            sing = ctx.enter_context(tc.tile_pool(name="sing", bufs=1))
            hpool = ctx.enter_context(tc.tile_pool(name="h", bufs=6, space="PSUM"))
            dxpool = ctx.enter_context(tc.tile_pool(name="dx", bufs=2, space="PSUM"))
            hsb = ctx.enter_context(tc.tile_pool(name="hsb", bufs=8))

            x16a = sing.tile([128, FB], f16)
            x16b = sing.tile([128, FB], f16)
            xof = sing.tile([128, 4096], f32)
            fire = sing.tile([128, 4096], f16)
            A = sing.tile([128, 384], f16)
            W2s = sing.tile([128, 32], f16)
            zeros = sing.tile([128, TW - SC_FD], f32)
            b1c = sing.tile([128, 1], f32)
            # dx-shift stacks: rows 24d+c = channel c shifted by dx=d-1;
            # row 72 = fire; rows 73..127 zero (pads K to 128 for full-rate
            # matmuls). One per group, ping-ponged per step.
            stk = [
                [sing.tile([128, FB], f16, name=f"stk_{g}_{b}") for b in range(2)]
                for g in range(G)
            ]

            # stacks + weights first (taps need only these), spread queues
            _ld = [nc.sync, nc.scalar, nc.gpsimd, nc.sync]
            for g in range(G):
                _ld[g].dma_start(
                    stk[g][0][:, 0:2376],
                    stkin[:, g * FB : g * FB + 2376],
                )
            for g in range(G):
                _ld[(g + 1) % 3].dma_start(
                    stk[g][0][:, 2376:FB],
                    stkin[:, g * FB + 2376 : (g + 1) * FB],
                )
            nc.scalar.dma_start(A[:], apack[:])
            nc.gpsimd.dma_start(W2s[:], w2p[:])
            nc.gpsimd.dma_start(b1c[:], b1col[:])
            nc.sync.dma_start(x16a[:], x16in[:])
            nc.scalar.dma_start(fire[:], firein[:])
            for g in range(G):
                # odd-step stacks: zero fire pad + K-padding rows (rows
                # 64..72 are rewritten by slice/fire DMAs every step)
                nc.gpsimd.memset(stk[g][1][64:128, :], 0.0)
            nc.gpsimd.memset(zeros[:], 0.0)

            xf16 = [x16a, x16b]
            fire3 = fire[:].rearrange("p (r w) -> p r w", w=128)
            xo3 = xof[:].rearrange("p (r w) -> p r w", w=128)
            # stack slice boundaries (flat y): rows 1-4 | 5-16 | 17-24 | 25-32
            SLICES = [(132, 660), (660, 2244), (2244, 3300), (3300, 4356)]
            ISSUE = None  # set per step

            def emit_slice(s, sl):
                """Stack-slice copies for step s+1 (wrap cols already set)."""
                x6 = xf16[(s + 1) % 2]
                ylo, yhi = SLICES[sl]
                for g in range(G):
                    sg = stk[g][(s + 1) % 2]
                    for d in range(3):
                        eng = ISSUE[(g + d) % len(ISSUE)]
                        eng.dma_start(
                            sg[24 * d : 24 * d + 24, ylo:yhi],
                            x6[32 * g : 32 * g + 24, ylo + d : yhi + d],
                        )

            def emit_fire(s):
                """Fire rows for step s+1 (no cast deps; only WAR on s-1)."""
                sb = (s + 1) % 2
                for g in range(G):
                    s3 = stk[g][sb][:].rearrange("p (r w) -> p r w", w=PITCH)
                    nc.gpsimd.dma_start(
                        s3[72:73, 1:33, 0:128],
                        fire3[32 * g + s + 1 : 32 * g + s + 2, :, :],
                    )

            def emit_fire_edges(s, top):
                """Neighbor-stack halo rows for step s+1. The top halo row
                (image row 32g-1) is neighbor (g-1)'s row 32 (slice 3, built
                last); the bottom halo (row 33) is neighbor (g+1)'s row 1
                (slice 0, built early)."""
                sb = (s + 1) % 2
                for g in range(G):
                    sg = stk[g][sb]
                    if top:
                        sm = stk[(g - 1) % G][sb]
                        nc.vector.tensor_copy(sg[:73, 0:132], sm[:73, 4224:4356])
                    else:
                        sp = stk[(g + 1) % G][sb]
                        nc.vector.tensor_copy(sg[:73, 4356:4488], sp[:73, 132:264])

            JORD = [2, 3, 1, 4, 0, 5, 6, 7]
            # stack slice sl becomes buildable once these adds are done
            TRIGGER = {0: 0, 1: 1, 5: 2, 7: 3}  # add j -> slice index
            for s in range(STEPS):
                ISSUE = [nc.sync, nc.gpsimd]
                last = s + 1 == STEPS
                if not last:
                    emit_fire(s)
                xc = xf16[s % 2][:].rearrange("p (r w) -> p r w", w=PITCH)
                xn = xf16[(s + 1) % 2][:].rearrange("p (r w) -> p r w", w=PITCH)
                stks = [
                    stk[g][s % 2][:].rearrange("p (r w) -> p r w", w=PITCH)
                    for g in range(G)
                ]

                def mm2_update(j, hss):
                    """mm2 (4x col-tiled, all hss already evacuated so the
                    four matmuls dispatch back-to-back and overlap) + the
                    residual add + wrap cols + stack-slice triggers."""
                    r0 = 4 * j + 1
                    dxt = dxpool.tile(
                        [128, TW], f32, tag="dx", name=f"dx_{s}_{j}"
                    )
                    for g in range(G):
                        nc.tensor.matmul(
                            dxt[32 * g : 32 * g + 32, :],
                            W2s[:],
                            hss[g][:],
                            start=True,
                            stop=True,
                            tile_position=(0, 32 * g),
                        )
                    dx3 = dxt[:].rearrange("p (a b) -> p a b", b=128)
                    if last:
                        # final step: accumulate straight into f32 output
                        nc.vector.tensor_tensor(
                            xo3[:, 4 * j : 4 * j + 4, :],
                            dx3,
                            xc[:, r0 : r0 + 4, 1:129],
                            Alu.add,
                        )
                        return
                    # x_next = x + dx  (dx rows 24..31 of each band are 0)
                    nc.vector.tensor_tensor(
                        xn[:, r0 : r0 + 4, 1:129],
                        dx3,
                        xc[:, r0 : r0 + 4, 1:129],
                        Alu.add,
                    )
                    # circular wrap cols: col 0 <- image col 127 (= col 128
                    # just written), col 129 <- image col 0 (= col 1). Tiny
                    # copies off the fresh master keep the dx PSUM bank's
                    # only reader the main add (early WAR release for mm2).
                    nc.vector.tensor_copy(
                        xn[:, r0 : r0 + 4, 0:1],
                        xn[:, r0 : r0 + 4, 128:129],
                    )
                    nc.vector.tensor_copy(
                        xn[:, r0 : r0 + 4, 129:130],
                        xn[:, r0 : r0 + 4, 1:2],
                    )
                    if j in TRIGGER:
                        emit_slice(s, TRIGGER[j])
                        if TRIGGER[j] == 0:
                            emit_fire_edges(s, top=False)

                prev = None
                for j in JORD:
                    r0 = 4 * j + 1
                    hss = []
                    # mm1: g-major, 3 dy-matmuls per group (K padded to 128
                    # for full rate); dy shift via the rhs AP, dx via the
                    # stack rows. Early per-group stop lets evacuation
                    # overlap the remaining groups' matmuls.
                    for g in range(G):
                        ht = hpool.tile(
                            [128, TW], f32, tag="h", name=f"h_{s}_{j}_{g}"
                        )
                        for di, dy in enumerate((-1, 0, 1)):
                            rhs = stks[g][0:128, r0 + dy : r0 + dy + 4, 0:128]
                            lhsT = A[0:128, 128 * (dy + 1) : 128 * (dy + 2)]
                            nc.tensor.matmul(
                                ht[:, :],
                                lhsT,
                                rhs,
                                start=(di == 0),
                                stop=(di == 2),
                                tile_position=(0, 0),
                            )
                        hs = hsb.tile(
                            [128, TW], f16, tag="hsb", name=f"hs_{s}_{j}_{g}"
                        )
                        # h' = relu(h + b1 - M); dy=0 fire row added fire*M
                        nc.scalar.activation(
                            hs[:, :SC_FD],
                            ht[:, :SC_FD],
                            Act.Relu,
                            bias=b1c[:],
                        )
                        nc.vector.scalar_tensor_tensor(
                            hs[:, SC_FD:],
                            ht[:, SC_FD:],
                            b1c[:],
                            zeros[:],
                            Alu.add,
                            Alu.max,
                        )
                        hss.append(hs)
                    if prev is not None:
                        mm2_update(*prev)
                    prev = (j, hss)
                mm2_update(*prev)
                if not last:
                    emit_fire_edges(s, top=True)

            nc.sync.dma_start(xout[:, :], xof[:, :])

    nc.compile()
    return nc


def _get_module():
    if "nc" not in _CACHE:
        _CACHE["nc"] = _build_module()
    return _CACHE["nc"]


def _prep_weights(w1, w2, W1, b1, W2):
    A = np.zeros((9, HID, C), np.float32)
    for t in range(9):
        dy, dxx = t // 3 - 1, t % 3 - 1
        A[t] = (
            W1[:, 24:48] * w1[dy + 1, dxx + 1, 0][None, :]
            + W1[:, 48:72] * w2[dy + 1, dxx + 1, 0][None, :]
        )
    A[4] += W1[:, :24]
    apack = np.zeros((128, 384), np.float32)
    for d in range(3):
        for dyi in range(3):
            t = dyi * 3 + d
            apack[24 * d : 24 * d + 24, 128 * dyi : 128 * dyi + 128] = A[t].T
    apack[72, 128:256] = M_FIRE
    w2pk = np.zeros((128, 32), np.float32)
    w2pk[:, NIC:C] = W2[NIC:C].T
    b1c = (b1 - M_FIRE).reshape(128, 1).astype(np.float32)
    return apack.astype(np.float16), w2pk.astype(np.float16), b1c


def _pack_x(ximg):
    """[128,128,24] image -> [128, FB] haloed channel-major fp16."""
    xin = np.zeros((128, FB), np.float32)
    cols = (np.arange(-1, 129)) % WID
    for g in range(G):
        rows = (np.arange(-1, 33) + 32 * g) % H
        blk = ximg[rows][:, cols, :]  # [34, 130, 24]
        buf = np.zeros((24, GROWS, PITCH), np.float32)
        buf[:, :, :130] = np.transpose(blk, (2, 0, 1))
        xin[32 * g : 32 * g + 24] = buf.reshape(24, FB)
    return xin.astype(np.float16)


def _unpack_x(xo):
    """[128, 4096] -> [128,128,24] image."""
    img = np.empty((H, WID, C), np.float32)
    for g in range(G):
        blk = xo[32 * g : 32 * g + 24].reshape(24, RG, WID)
        img[32 * g : 32 * g + 32] = np.transpose(blk, (1, 2, 0))
    return img


def _build_stack0(x16, fire16):
    """Host: step-0 stacks, one [128, FB] block per group."""
    stkin = np.zeros((128, 4 * FB), np.float16)
    for g in range(G):
        blk = stkin[:, g * FB : (g + 1) * FB]
        for d in range(3):
            blk[24 * d : 24 * d + 24, : FB - d] = x16[
                32 * g : 32 * g + 24, d : FB
            ]
        fr = fire16[32 * g].reshape(32, 128)
        f2 = blk[72].reshape(GROWS, PITCH)
        f2[1:33, 0:128] = fr
    return stkin


def _make_in_maps(x, w1, w2, W1, b1, W2, rand_u):
    apack, w2pk, b1c = _prep_weights(w1, w2, W1, b1, W2)
    in_maps = []
    for b in range(B):
        u = rand_u[:, b, :, :, 0].reshape(STEPS, H * WID)
        fire16 = np.zeros((128, 4096), np.float16)
        for g in range(G):
            for s in range(STEPS):
                fire16[32 * g + s] = (
                    u[s, g * 4096 : (g + 1) * 4096] < 0.5
                ).astype(np.float16)
        x16 = _pack_x(np.asarray(x[b], np.float32))
        in_maps.append(
            {
                "apack": apack,
                "w2p": w2pk,
                "b1col": b1c,
                "x16in": x16,
                "firein": fire16,
                "stkin": _build_stack0(x16, fire16),
            }
        )
    return in_maps


def kernel(x, w1, w2, W1, b1, W2, rand_u, steps, **kw):
    from concourse.bass_utils import run_bass_kernel_spmd

    assert int(steps) == STEPS
    x = np.asarray(x, np.float32)
    in_maps = _make_in_maps(
        x,
        np.asarray(w1, np.float32),
        np.asarray(w2, np.float32),
        np.asarray(W1, np.float32),
        np.asarray(b1, np.float32),
        np.asarray(W2, np.float32),
        np.asarray(rand_u, np.float32),
    )
    nc = _get_module()
    res = run_bass_kernel_spmd(nc, in_maps, core_ids=list(range(B)))
    _CACHE["last_results"] = res
    out = np.empty((B, H, WID, C), np.float32)
    for b in range(B):
        out[b] = _unpack_x(res.results[b]["xout"])
    return out


# revision 10
# speedup vs baseline: 1.0015x; 1.0015x over previous
"""Trainium2 Bass kernel for nn_BasicNCAModel (neural cellular automaton).

Sharding: data-parallel over batch B=8 across 8 NeuronCores (1 image/core).
kernel() takes full inputs, shards per image on the host, runs the SPMD Bass
module via run_bass_kernel_spmd (PJRT under axon), and reassembles.

Per-core design (hardcoded for B=8, H=W=128, C=24, hidden=128, steps=8):
  - x lives ONLY as an fp16 master (ping-ponged per step), channel-major
    with a halo: partition 32g+c holds channel c of image rows
    [32g-1, 32g+32] (4 row-groups, 34 rows x 132 pitch), so circular
    padding becomes plain address offsets. fp16 rounding of the residual
    accumulates ~1e-3 rel over 8 steps (tolerance 2e-2).
  - perceive + W1 fuse into per-tap matrices A_t[k,c] = W1[k,24+c]*w1[t,c]
    + W1[k,48+c]*w2[t,c] (+W1[k,c] at the center tap). Per group a fp16
    "dx-stack" holds rows 24d+c = x16 shifted by dx=d-1 (shift baked into
    a contiguous DMA copy), row 72 = fire = (u<0.5) (host-precomputed),
    rows 73..127 = zero so K=128 (full-K matmuls run ~2x faster than
    partial-K). mm1 is 3 matmuls per 512-pixel tile (dy in {-1,0,1} via
    +-PITCH in the rhs AP), g-major so each group's PSUM bank stops early;
    the center one adds fire*M (M=512). Evacuation h'=relu(h+b1-M) is
    split ScalarE (cols 0:352, relu+bias) / VectorE (cols 352:512, STT
    add+max): masked pixels get relu(h-M)=0 so dx=0 exactly (fire trick).
  - mm2 (fp16, 4x col-tiled via tile_position (0,32g)) issues immediately
    after the group's evacuation: group g's dx lands at PSUM partitions
    32g..32g+31 (W2^T with channels 0..3 zeroed for the channel mask),
    so x16' = x16 + dx is one full-width DVE add per tile (f16 out). Two
    tiny adds write the circular wrap cols 0/129 straight from dx+x (no
    separate halo copies). The last step adds into an f32 staging tile
    that DMAs to the output. Stack rebuild DMAs are sliced and issued as
    soon as the adds they need complete (j-order [2,3,1,4,0,5,6,7] keeps
    cross-step dependencies off the tensor-engine critical path); stack
    halo rows come from the neighbor group's stack (DVE copies, top half
    mid-step, bottom half at step end).
"""

import sys

if "/opt/trn_rl_repo" not in sys.path:
    sys.path.insert(0, "/opt/trn_rl_repo")

import numpy as np

C = 24
NIC = 4
H = 128
WID = 128
HID = 128
STEPS = 8
B = 8
G = 4          # row groups
RG = 32        # image rows per group
PITCH = 132    # free-dim row pitch (130 used + 2 pad)
GROWS = 34     # rows incl halo
FB = GROWS * PITCH
TW = 512       # pixel tile = 4 image rows * 128 cols
JT = RG // 4   # tiles per group per step
M_FIRE = 512.0
SC_FD = 352    # ScalarE share of h' evacuation (cols 0:SC_FD)

_CACHE = {}


def _build_module():
    from concourse import bacc, mybir, tile

    f32 = mybir.dt.float32
    f16 = mybir.dt.float16
    Alu = mybir.AluOpType
    Act = mybir.ActivationFunctionType

    nc = bacc.Bacc(
        "TRN2",
        target_bir_lowering=False,
        debug=False,
        enable_asserts=False,
        num_devices=8,
    )

    apack = nc.dram_tensor("apack", [128, 384], f16, kind="ExternalInput").ap()
    w2p = nc.dram_tensor("w2p", [128, 32], f16, kind="ExternalInput").ap()
    b1col = nc.dram_tensor("b1col", [128, 1], f32, kind="ExternalInput").ap()
    x16in = nc.dram_tensor("x16in", [128, FB], f16, kind="ExternalInput").ap()
    firein = nc.dram_tensor("firein", [128, 4096], f16, kind="ExternalInput").ap()
    stkin = nc.dram_tensor("stkin", [128, 4 * FB], f16, kind="ExternalInput").ap()
    xout = nc.dram_tensor("xout", [128, 4096], f32, kind="ExternalOutput").ap()

    with tile.TileContext(nc) as tc:
        import contextlib

        with contextlib.ExitStack() as ctx:
            sing = ctx.enter_context(tc.tile_pool(name="sing", bufs=1))
            hpool = ctx.enter_context(tc.tile_pool(name="h", bufs=6, space="PSUM"))
            dxpool = ctx.enter_context(tc.tile_pool(name="dx", bufs=2, space="PSUM"))
            hsb = ctx.enter_context(tc.tile_pool(name="hsb", bufs=8))

            x16a = sing.tile([128, FB], f16)
            x16b = sing.tile([128, FB], f16)
            xof = sing.tile([128, 4096], f32)
            fire = sing.tile([128, 4096], f16)
            A = sing.tile([128, 384], f16)
            W2s = sing.tile([128, 32], f16)
            zeros = sing.tile([128, TW - SC_FD], f32)
            b1c = sing.tile([128, 1], f32)
            # dx-shift stacks: rows 24d+c = channel c shifted by dx=d-1;
            # row 72 = fire; rows 73..127 zero (pads K to 128 for full-rate
            # matmuls). One per group, ping-ponged per step.
            stk = [
                [sing.tile([128, FB], f16, name=f"stk_{g}_{b}") for b in range(2)]
                for g in range(G)
            ]

            # stacks + weights first (taps need only these); one queue per
            # group prefix so the first j-tiles' rows land in parallel
            nc.scalar.dma_start(A[:], apack[:])
            _ld = [nc.sync, nc.scalar, nc.gpsimd, nc.sync]
            for g in range(G):
                _ld[g].dma_start(
                    stk[g][0][:, 0:2376],
                    stkin[:, g * FB : g * FB + 2376],
                )
            for g in range(G):
                _ld[(g + 1) % 3].dma_start(
                    stk[g][0][:, 2376:FB],
                    stkin[:, g * FB + 2376 : (g + 1) * FB],
                )
            nc.gpsimd.dma_start(W2s[:], w2p[:])
            nc.gpsimd.dma_start(b1c[:], b1col[:])
            nc.sync.dma_start(x16a[:], x16in[:])
            nc.scalar.dma_start(fire[:], firein[:])
            for g in range(G):
                # odd-step stacks: zero fire pad + K-padding rows (rows
                # 64..72 are rewritten by slice/fire DMAs every step)
                nc.gpsimd.memset(stk[g][1][64:128, :], 0.0)
            nc.gpsimd.memset(zeros[:], 0.0)

            xf16 = [x16a, x16b]
            fire3 = fire[:].rearrange("p (r w) -> p r w", w=128)
            xo3 = xof[:].rearrange("p (r w) -> p r w", w=128)
            # stack slice boundaries (flat y): rows 1-4 | 5-16 | 17-24 | 25-32
            SLICES = [(132, 660), (660, 2244), (2244, 3300), (3300, 4356)]
            ISSUE = None  # set per step

            def emit_slice(s, sl):
                """Stack-slice copies for step s+1 (wrap cols already set)."""
                x6 = xf16[(s + 1) % 2]
                ylo, yhi = SLICES[sl]
                for g in range(G):
                    sg = stk[g][(s + 1) % 2]
                    for d in range(3):
                        eng = ISSUE[(g + d) % len(ISSUE)]
                        eng.dma_start(
                            sg[24 * d : 24 * d + 24, ylo:yhi],
                            x6[32 * g : 32 * g + 24, ylo + d : yhi + d],
                        )

            def emit_fire(s):
                """Fire rows for step s+1 (no cast deps; only WAR on s-1)."""
                sb = (s + 1) % 2
                for g in range(G):
                    s3 = stk[g][sb][:].rearrange("p (r w) -> p r w", w=PITCH)
                    nc.gpsimd.dma_start(
                        s3[72:73, 1:33, 0:128],
                        fire3[32 * g + s + 1 : 32 * g + s + 2, :, :],
                    )

            def emit_fire_edges(s, top):
                """Neighbor-stack halo rows for step s+1. The top halo row
                (image row 32g-1) is neighbor (g-1)'s row 32 (slice 3, built
                last); the bottom halo (row 33) is neighbor (g+1)'s row 1
                (slice 0, built early)."""
                sb = (s + 1) % 2
                for g in range(G):
                    sg = stk[g][sb]
                    # on GpSimd: keeps these off the DVE queue head (they
                    # wait on slice DMAs and would block the next step's
                    # evacuation ops). 80 partitions (alignment): rows
                    # 73..79 are K-pad zeros on both sides.
                    if top:
                        sm = stk[(g - 1) % G][sb]
                        nc.gpsimd.tensor_copy(sg[:80, 0:132], sm[:80, 4224:4356])
                    else:
                        sp = stk[(g + 1) % G][sb]
                        nc.gpsimd.tensor_copy(sg[:80, 4356:4488], sp[:80, 132:264])

            JORD = [2, 3, 1, 4, 0, 5, 6, 7]
            # stack slice sl becomes buildable once these adds are done
            TRIGGER = {0: 0, 1: 1, 5: 2, 7: 3}  # add j -> slice index
            for s in range(STEPS):
                ISSUE = [nc.sync, nc.gpsimd]
                last = s + 1 == STEPS
                if not last:
                    emit_fire(s)
                xc = xf16[s % 2][:].rearrange("p (r w) -> p r w", w=PITCH)
                xn = xf16[(s + 1) % 2][:].rearrange("p (r w) -> p r w", w=PITCH)
                stks = [
                    stk[g][s % 2][:].rearrange("p (r w) -> p r w", w=PITCH)
                    for g in range(G)
                ]

                def mm2_update(j, hss):
                    """mm2 (4x col-tiled, all hss already evacuated so the
                    four matmuls dispatch back-to-back and overlap) + the
                    residual add + wrap cols + stack-slice triggers."""
                    r0 = 4 * j + 1
                    dxt = dxpool.tile(
                        [128, TW], f32, tag="dx", name=f"dx_{s}_{j}"
                    )
                    for g in range(G):
                        nc.tensor.matmul(
                            dxt[32 * g : 32 * g + 32, :],
                            W2s[:],
                            hss[g][:],
                            start=True,
                            stop=True,
                            tile_position=(0, 32 * g),
                        )
                    dx3 = dxt[:].rearrange("p (a b) -> p a b", b=128)
                    if last:
                        # final step: accumulate straight into f32 output
                        nc.vector.tensor_tensor(
                            xo3[:, 4 * j : 4 * j + 4, :],
                            dx3,
                            xc[:, r0 : r0 + 4, 1:129],
                            Alu.add,
                        )
                        return
                    # x_next = x + dx  (dx rows 24..31 of each band are 0)
                    nc.vector.tensor_tensor(
                        xn[:, r0 : r0 + 4, 1:129],
                        dx3,
                        xc[:, r0 : r0 + 4, 1:129],
                        Alu.add,
                    )
                    # circular wrap cols: col 0 <- image col 127 (= col 128
                    # just written), col 129 <- image col 0 (= col 1). Tiny
                    # copies off the fresh master keep the dx PSUM bank's
                    # only reader the main add (early WAR release for mm2).
                    nc.vector.tensor_copy(
                        xn[:, r0 : r0 + 4, 0:1],
                        xn[:, r0 : r0 + 4, 128:129],
                    )
                    nc.vector.tensor_copy(
                        xn[:, r0 : r0 + 4, 129:130],
                        xn[:, r0 : r0 + 4, 1:2],
                    )
                    if j in TRIGGER:
                        emit_slice(s, TRIGGER[j])
                        if TRIGGER[j] == 0:
                            emit_fire_edges(s, top=False)

                prev = None
                for j in JORD:
                    r0 = 4 * j + 1
                    hss = []
                    # mm1: g-major, 3 dy-matmuls per group (K padded to 128
                    # for full rate); dy shift via the rhs AP, dx via the
                    # stack rows. Early per-group stop lets evacuation
                    # overlap the remaining groups' matmuls.
                    for g in range(G):
                        ht = hpool.tile(
                            [128, TW], f32, tag="h", name=f"h_{s}_{j}_{g}"
                        )
                        for di, dy in enumerate((-1, 0, 1)):
                            rhs = stks[g][0:128, r0 + dy : r0 + dy + 4, 0:128]
                            lhsT = A[0:128, 128 * (dy + 1) : 128 * (dy + 2)]
                            nc.tensor.matmul(
                                ht[:, :],
                                lhsT,
                                rhs,
                                start=(di == 0),
                                stop=(di == 2),
                                tile_position=(0, 0),
                            )
                        hs = hsb.tile(
                            [128, TW], f16, tag="hsb", name=f"hs_{s}_{j}_{g}"
                        )
                        # h' = relu(h + b1 - M); dy=0 fire row added fire*M
                        nc.scalar.activation(
                            hs[:, :SC_FD],
                            ht[:, :SC_FD],
                            Act.Relu,
                            bias=b1c[:],
                        )
                        nc.vector.scalar_tensor_tensor(
                            hs[:, SC_FD:],
                            ht[:, SC_FD:],
                            b1c[:],
                            zeros[:],
                            Alu.add,
                            Alu.max,
                        )
                        hss.append(hs)
                    if prev is not None:
                        mm2_update(*prev)
                    prev = (j, hss)
                mm2_update(*prev)
                if not last:
                    emit_fire_edges(s, top=True)

            nc.sync.dma_start(xout[:, :], xof[:, :])

    nc.compile()
    return nc


def _get_module():
    if "nc" not in _CACHE:
        _CACHE["nc"] = _build_module()
    return _CACHE["nc"]


def _prep_weights(w1, w2, W1, b1, W2):
    A = np.zeros((9, HID, C), np.float32)
    for t in range(9):
        dy, dxx = t // 3 - 1, t % 3 - 1
        A[t] = (
            W1[:, 24:48] * w1[dy + 1, dxx + 1, 0][None, :]
            + W1[:, 48:72] * w2[dy + 1, dxx + 1, 0][None, :]
        )
    A[4] += W1[:, :24]
    apack = np.zeros((128, 384), np.float32)
    for d in range(3):
        for dyi in range(3):
            t = dyi * 3 + d
            apack[24 * d : 24 * d + 24, 128 * dyi : 128 * dyi + 128] = A[t].T
    apack[72, 128:256] = M_FIRE
    w2pk = np.zeros((128, 32), np.float32)
    w2pk[:, NIC:C] = W2[NIC:C].T
    b1c = (b1 - M_FIRE).reshape(128, 1).astype(np.float32)
    return apack.astype(np.float16), w2pk.astype(np.float16), b1c


def _pack_x(ximg):
    """[128,128,24] image -> [128, FB] haloed channel-major fp16."""
    xin = np.zeros((128, FB), np.float32)
    cols = (np.arange(-1, 129)) % WID
    for g in range(G):
        rows = (np.arange(-1, 33) + 32 * g) % H
        blk = ximg[rows][:, cols, :]  # [34, 130, 24]
        buf = np.zeros((24, GROWS, PITCH), np.float32)
        buf[:, :, :130] = np.transpose(blk, (2, 0, 1))
        xin[32 * g : 32 * g + 24] = buf.reshape(24, FB)
    return xin.astype(np.float16)


def _unpack_x(xo):
    """[128, 4096] -> [128,128,24] image."""
    img = np.empty((H, WID, C), np.float32)
    for g in range(G):
        blk = xo[32 * g : 32 * g + 24].reshape(24, RG, WID)
        img[32 * g : 32 * g + 32] = np.transpose(blk, (1, 2, 0))
    return img


def _build_stack0(x16, fire16):
    """Host: step-0 stacks, one [128, FB] block per group."""
    stkin = np.zeros((128, 4 * FB), np.float16)
    for g in range(G):
        blk = stkin[:, g * FB : (g + 1) * FB]
        for d in range(3):
            blk[24 * d : 24 * d + 24, : FB - d] = x16[
                32 * g : 32 * g + 24, d : FB
            ]
        fr = fire16[32 * g].reshape(32, 128)
        f2 = blk[72].reshape(GROWS, PITCH)
        f2[1:33, 0:128] = fr
    return stkin


def _make_in_maps(x, w1, w2, W1, b1, W2, rand_u):
    apack, w2pk, b1c = _prep_weights(w1, w2, W1, b1, W2)
    in_maps = []
    for b in range(B):
        u = rand_u[:, b, :, :, 0].reshape(STEPS, H * WID)
        fire16 = np.zeros((128, 4096), np.float16)
        for g in range(G):
            for s in range(STEPS):
                fire16[32 * g + s] = (
                    u[s, g * 4096 : (g + 1) * 4096] < 0.5
                ).astype(np.float16)
        x16 = _pack_x(np.asarray(x[b], np.float32))
        in_maps.append(
            {
                "apack": apack,
                "w2p": w2pk,
                "b1col": b1c,
                "x16in": x16,
                "firein": fire16,
                "stkin": _build_stack0(x16, fire16),
            }
        )
    return in_maps


def kernel(x, w1, w2, W1, b1, W2, rand_u, steps, **kw):
    from concourse.bass_utils import run_bass_kernel_spmd

    assert int(steps) == STEPS
    x = np.asarray(x, np.float32)
    in_maps = _make_in_maps(
        x,
        np.asarray(w1, np.float32),
        np.asarray(w2, np.float32),
        np.asarray(W1, np.float32),
        np.asarray(b1, np.float32),
        np.asarray(W2, np.float32),
        np.asarray(rand_u, np.float32),
    )
    nc = _get_module()
    res = run_bass_kernel_spmd(nc, in_maps, core_ids=list(range(B)))
    _CACHE["last_results"] = res
    out = np.empty((B, H, WID, C), np.float32)
    for b in range(B):
        out[b] = _unpack_x(res.results[b]["xout"])
    return out


# revision 13
# speedup vs baseline: 1.0380x; 1.0365x over previous
"""Trainium2 Bass kernel for nn_BasicNCAModel (neural cellular automaton).

Sharding: data-parallel over batch B=8 across 8 NeuronCores (1 image/core).
kernel() takes full inputs, shards per image on the host, runs the SPMD Bass
module via run_bass_kernel_spmd (PJRT under axon), and reassembles.

Per-core design (hardcoded for B=8, H=W=128, C=24, hidden=128, steps=8):
  - x lives ONLY as an fp16 master (ping-ponged per step), channel-major
    with per-group halos: partition 32g+c holds channel c of image rows
    [32g-1, 32g+32] (4 row-groups, 34 rows x 132 pitch) so circular
    padding becomes plain address offsets.
  - One UNIFIED dx-shift stack per step parity: partitions 24d+c hold
    channel c shifted by dx=d-1, free dim = 130 stack-rows x 132 (row i =
    image row i-1; rows 0/129 are the circular halo rows, refreshed by two
    tiny same-partition copies per step); partition 72 = fire = (u<0.5)
    (host-precomputed), partitions 73..127 zero so K=128. Group boundaries
    are just adjacent rows, so there are no per-group halo exchanges.
  - perceive + W1 fuse into per-tap matrices A_t[k,c] = W1[k,24+c]*w1[t,c]
    + W1[k,48+c]*w2[t,c] (+W1[k,c] at the center tap). mm1 is 3 matmuls
    per 512-pixel tile (dy in {-1,0,1} via +-132 in the rhs AP), g-major;
    the center one adds fire*M (M=512). Evacuation h'=relu(h+b1-M) splits
    ScalarE (cols 0:352, relu+bias) / VectorE (cols 352:512, STT add+max):
    masked pixels get relu(h-M)=0 so dx=0 exactly (fire trick).
  - mm2 (fp16, 4x col-tiled via tile_position (0,32g)) runs one j-tile
    behind mm1 -- carried ACROSS step boundaries -- so all four hss are
    evacuated by dispatch time and the four matmuls overlap in the PE
    array (measured ~4ns apart). Group g's dx lands at PSUM partitions
    32g..32g+31 (W2^T with channels 0..3 zeroed for the channel mask), so
    x16' = x16 + dx is one full-width DVE add per tile (f16 out), plus two
    tiny copies for the master's wrap cols. The last step adds into an f32
    staging tile that DMAs to the output. Stack rebuild DMAs are sliced
    and issued as soon as the adds they need complete (j-order
    [2,3,1,4,0,5,6,7] keeps cross-step dependencies off the tensor-engine
    critical path).
"""

import sys

if "/opt/trn_rl_repo" not in sys.path:
    sys.path.insert(0, "/opt/trn_rl_repo")

import numpy as np

C = 24
NIC = 4
H = 128
WID = 128
HID = 128
STEPS = 8
B = 8
G = 4          # row groups
RG = 32        # image rows per group
PITCH = 132    # free-dim row pitch (130 used + 2 pad)
GROWS = 34     # master rows per group incl halo
FB = GROWS * PITCH
SROWS = 130    # unified stack rows: halo + 128 + halo
SFL = SROWS * PITCH
TW = 512       # pixel tile = 4 image rows * 128 cols
M_FIRE = 512.0
SC_FD = 352    # ScalarE share of h' evacuation (cols 0:SC_FD)

_CACHE = {}


def _build_module():
    from concourse import bacc, mybir, tile

    f32 = mybir.dt.float32
    f16 = mybir.dt.float16
    Alu = mybir.AluOpType
    Act = mybir.ActivationFunctionType

    nc = bacc.Bacc(
        "TRN2",
        target_bir_lowering=False,
        debug=False,
        enable_asserts=False,
        num_devices=8,
    )

    apack = nc.dram_tensor("apack", [128, 384], f16, kind="ExternalInput").ap()
    w2p = nc.dram_tensor("w2p", [128, 32], f16, kind="ExternalInput").ap()
    b1col = nc.dram_tensor("b1col", [128, 1], f32, kind="ExternalInput").ap()
    x16in = nc.dram_tensor("x16in", [128, FB], f16, kind="ExternalInput").ap()
    firein = nc.dram_tensor("firein", [128, 4096], f16, kind="ExternalInput").ap()
    stkin = nc.dram_tensor("stkin", [128, SFL], f16, kind="ExternalInput").ap()
    xout = nc.dram_tensor("xout", [128, 4096], f32, kind="ExternalOutput").ap()

    with tile.TileContext(nc) as tc:
        import contextlib

        with contextlib.ExitStack() as ctx:
            sing = ctx.enter_context(tc.tile_pool(name="sing", bufs=1))
            hpool = ctx.enter_context(tc.tile_pool(name="h", bufs=6, space="PSUM"))
            dxpool = ctx.enter_context(tc.tile_pool(name="dx", bufs=2, space="PSUM"))
            hsb = ctx.enter_context(tc.tile_pool(name="hsb", bufs=8))

            x16a = sing.tile([128, FB], f16)
            x16b = sing.tile([128, FB], f16)
            xof = sing.tile([128, 4096], f32)
            fire = sing.tile([128, 4096], f16)
            A = sing.tile([128, 384], f16)
            W2s = sing.tile([128, 32], f16)
            zeros = sing.tile([128, TW - SC_FD], f32)
            b1c = sing.tile([128, 1], f32)
            stk = [sing.tile([128, SFL], f16, name=f"stk_{b}") for b in range(2)]

            # weights + the stack rows the first j-tiles read come first;
            # per-group prefixes (srows 32g+4 .. 32g+21 cover JORD 1,2,3,4)
            # spread across the three DMA-capable queues
            nc.scalar.dma_start(A[:], apack[:])
            QS = [nc.sync, nc.gpsimd, nc.sync, nc.scalar]
            for g in range(G):
                lo, hi = (32 * g + 4) * PITCH, (32 * g + 22) * PITCH
                QS[g].dma_start(stk[0][:, lo:hi], stkin[:, lo:hi])
            gaps = [(0, 4), (22, 36), (54, 68), (86, 100), (118, 130)]
            for k, (a, b) in enumerate(gaps):
                QS[k % 3].dma_start(
                    stk[0][:, a * PITCH : b * PITCH],
                    stkin[:, a * PITCH : b * PITCH],
                )
            nc.gpsimd.dma_start(W2s[:], w2p[:])
            nc.gpsimd.dma_start(b1c[:], b1col[:])
            nc.sync.dma_start(x16a[:], x16in[:])
            nc.scalar.dma_start(fire[:], firein[:])
            # odd-parity stack: zero fire pad + K-pad partitions once
            # (partitions 64..71 are rewritten by slice DMAs every step)
            nc.gpsimd.memset(stk[1][64:128, :], 0.0)
            nc.gpsimd.memset(zeros[:], 0.0)

            xf16 = [x16a, x16b]
            fire3 = fire[:].rearrange("p (r w) -> p r w", w=128)
            xo3 = xof[:].rearrange("p (r w) -> p r w", w=128)
            stkv = [stk[b][:].rearrange("p (r w) -> p r w", w=PITCH) for b in range(2)]
            # slice events: local image rows 0-3 | 4-15 | 16-23 | 24-31
            SLICES = [(0, 4), (4, 16), (16, 24), (24, 32)]
            JORD = [2, 3, 1, 4, 0, 5, 6, 7]
            TRIGGER = {0: 0, 1: 1, 5: 2, 7: 3}  # add j -> slice index

            def emit_slice(s, sl):
                """Stack-slice copies for step s+1 (wrap cols already in
                the master). Unified stack: dst srows 32g+1+lr."""
                x6 = xf16[(s + 1) % 2]
                sb = stk[(s + 1) % 2]
                la, lb = SLICES[sl]
                for g in range(G):
                    dlo = (32 * g + 1 + la) * PITCH
                    dhi = (32 * g + 1 + lb) * PITCH
                    slo = (la + 1) * PITCH
                    shi = (lb + 1) * PITCH
                    for d in range(3):
                        eng = [nc.sync, nc.gpsimd][(g + d) % 2]
                        eng.dma_start(
                            sb[24 * d : 24 * d + 24, dlo:dhi],
                            x6[32 * g : 32 * g + 24, slo + d : shi + d],
                        )

            def emit_fire(s):
                """Fire rows for step s+1 (static input; only WAR on s-1)."""
                sb = (s + 1) % 2
                for g in range(G):
                    nc.gpsimd.dma_start(
                        stkv[sb][72:73, 32 * g + 1 : 32 * g + 33, 0:128],
                        fire3[32 * g + s + 1 : 32 * g + s + 2, :, :],
                    )

            state = {"prev": None}

            def mm2_update(s, j, hss, last):
                """mm2 (4x col-tiled; hss all evacuated so the four matmuls
                dispatch back-to-back and overlap) + residual add + wrap
                cols + stack-slice triggers for step s+1."""
                r0 = 4 * j + 1
                xc = xf16[s % 2][:].rearrange("p (r w) -> p r w", w=PITCH)
                xn = xf16[(s + 1) % 2][:].rearrange("p (r w) -> p r w", w=PITCH)
                dxt = dxpool.tile([128, TW], f32, tag="dx", name=f"dx_{s}_{j}")
                for g in range(G):
                    nc.tensor.matmul(
                        dxt[32 * g : 32 * g + 32, :],
                        W2s[:],
                        hss[g][:],
                        start=True,
                        stop=True,
                        tile_position=(0, 32 * g),
                    )
                dx3 = dxt[:].rearrange("p (a b) -> p a b", b=128)
                if last:
                    # final step: accumulate straight into f32 output
                    nc.vector.tensor_tensor(
                        xo3[:, 4 * j : 4 * j + 4, :],
                        dx3,
                        xc[:, r0 : r0 + 4, 1:129],
                        Alu.add,
                    )
                    return
                # x_next = x + dx  (dx rows 24..31 of each band are 0)
                nc.vector.tensor_tensor(
                    xn[:, r0 : r0 + 4, 1:129],
                    dx3,
                    xc[:, r0 : r0 + 4, 1:129],
                    Alu.add,
                )
                # master wrap cols: col 0 <- col 128, col 129 <- col 1
                nc.vector.tensor_copy(
                    xn[:, r0 : r0 + 4, 0:1], xn[:, r0 : r0 + 4, 128:129]
                )
                nc.vector.tensor_copy(
                    xn[:, r0 : r0 + 4, 129:130], xn[:, r0 : r0 + 4, 1:2]
                )
                if j in TRIGGER:
                    emit_slice(s, TRIGGER[j])
                    # wrap-row copies on GpSimd: they wait on slice DMAs
                    # and would head-block the DVE queue. Partitions 73..79
                    # are K-pad zeros (alignment padding).
                    nb = stk[(s + 1) % 2]
                    if TRIGGER[j] == 0:
                        # stack halo row 129 (image row 0) <- srow 1
                        nc.gpsimd.tensor_copy(
                            nb[:80, 129 * PITCH : 130 * PITCH],
                            nb[:80, 1 * PITCH : 2 * PITCH],
                        )
                    if TRIGGER[j] == 3:
                        # stack halo row 0 (image row 127) <- srow 128
                        nc.gpsimd.tensor_copy(
                            nb[:80, 0:PITCH],
                            nb[:80, 128 * PITCH : 129 * PITCH],
                        )

            for s in range(STEPS):
                last = s + 1 == STEPS
                if not last:
                    emit_fire(s)
                sv = stkv[s % 2]
                for j in JORD:
                    hss = []
                    # mm1: g-major, 3 dy-matmuls per group (K padded to 128
                    # for full rate); dy shift via the rhs AP srow offset,
                    # dx via the stack partitions. Early per-group stop
                    # lets evacuation overlap the remaining groups.
                    for g in range(G):
                        sr0 = 32 * g + 4 * j + 1
                        ht = hpool.tile(
                            [128, TW], f32, tag="h", name=f"h_{s}_{j}_{g}"
                        )
                        for di, dy in enumerate((-1, 0, 1)):
                            rhs = sv[0:128, sr0 + dy : sr0 + dy + 4, 0:128]
                            lhsT = A[0:128, 128 * (dy + 1) : 128 * (dy + 2)]
                            nc.tensor.matmul(
                                ht[:, :],
                                lhsT,
                                rhs,
                                start=(di == 0),
                                stop=(di == 2),
                                tile_position=(0, 0),
                            )
                        hs = hsb.tile(
                            [128, TW], f16, tag="hsb", name=f"hs_{s}_{j}_{g}"
                        )
                        # h' = relu(h + b1 - M); dy=0 fire row added fire*M
                        nc.scalar.activation(
                            hs[:, :SC_FD],
                            ht[:, :SC_FD],
                            Act.Relu,
                            bias=b1c[:],
                        )
                        nc.vector.scalar_tensor_tensor(
                            hs[:, SC_FD:],
                            ht[:, SC_FD:],
                            b1c[:],
                            zeros[:],
                            Alu.add,
                            Alu.max,
                        )
                        hss.append(hs)
                    if state["prev"] is not None:
                        mm2_update(*state["prev"])
                    state["prev"] = (s, j, hss, last)
            mm2_update(*state["prev"])

            nc.sync.dma_start(xout[:, :], xof[:, :])

    nc.compile()
    return nc


def _get_module():
    if "nc" not in _CACHE:
        _CACHE["nc"] = _build_module()
    return _CACHE["nc"]


def _prep_weights(w1, w2, W1, b1, W2):
    A = np.zeros((9, HID, C), np.float32)
    for t in range(9):
        dy, dxx = t // 3 - 1, t % 3 - 1
        A[t] = (
            W1[:, 24:48] * w1[dy + 1, dxx + 1, 0][None, :]
            + W1[:, 48:72] * w2[dy + 1, dxx + 1, 0][None, :]
        )
    A[4] += W1[:, :24]
    apack = np.zeros((128, 384), np.float32)
    for d in range(3):
        for dyi in range(3):
            t = dyi * 3 + d
            apack[24 * d : 24 * d + 24, 128 * dyi : 128 * dyi + 128] = A[t].T
    apack[72, 128:256] = M_FIRE
    w2pk = np.zeros((128, 32), np.float32)
    w2pk[:, NIC:C] = W2[NIC:C].T
    b1c = (b1 - M_FIRE).reshape(128, 1).astype(np.float32)
    return apack.astype(np.float16), w2pk.astype(np.float16), b1c


def _pack_x(ximg):
    """[128,128,24] image -> [128, FB] haloed channel-major fp16."""
    xin = np.zeros((128, FB), np.float32)
    cols = (np.arange(-1, 129)) % WID
    for g in range(G):
        rows = (np.arange(-1, 33) + 32 * g) % H
        blk = ximg[rows][:, cols, :]  # [34, 130, 24]
        buf = np.zeros((24, GROWS, PITCH), np.float32)
        buf[:, :, :130] = np.transpose(blk, (2, 0, 1))
        xin[32 * g : 32 * g + 24] = buf.reshape(24, FB)
    return xin.astype(np.float16)


def _unpack_x(xo):
    """[128, 4096] -> [128,128,24] image."""
    img = np.empty((H, WID, C), np.float32)
    for g in range(G):
        blk = xo[32 * g : 32 * g + 24].reshape(24, RG, WID)
        img[32 * g : 32 * g + 32] = np.transpose(blk, (1, 2, 0))
    return img


def _build_stack0(ximg, fire0):
    """Host: step-0 unified stack [128, SFL] (srow i = image row i-1)."""
    stk0 = np.zeros((128, SROWS, PITCH), np.float32)
    rows = np.arange(-1, 129) % H  # srow i -> image row
    for d in range(3):
        # device convention: stack col q (block d) = image col q + d - 1
        cols = (np.arange(0, 130) + (d - 1)) % WID
        blk = ximg[rows][:, cols, :]  # [130, 130, 24]
        stk0[24 * d : 24 * d + 24, :, :130] = np.transpose(blk, (2, 0, 1))
    stk0[72, :, 0:128] = fire0[rows]
    return stk0.reshape(128, SFL).astype(np.float16)


def _make_in_maps(x, w1, w2, W1, b1, W2, rand_u):
    apack, w2pk, b1c = _prep_weights(w1, w2, W1, b1, W2)
    in_maps = []
    for b in range(B):
        u = rand_u[:, b, :, :, 0].reshape(STEPS, H * WID)
        fire16 = np.zeros((128, 4096), np.float16)
        for g in range(G):
            for s in range(STEPS):
                fire16[32 * g + s] = (
                    u[s, g * 4096 : (g + 1) * 4096] < 0.5
                ).astype(np.float16)
        ximg = np.asarray(x[b], np.float32)
        fire0 = (u[0].reshape(H, WID) < 0.5).astype(np.float32)
        in_maps.append(
            {
                "apack": apack,
                "w2p": w2pk,
                "b1col": b1c,
                "x16in": _pack_x(ximg),
                "firein": fire16,
                "stkin": _build_stack0(ximg, fire0),
            }
        )
    return in_maps


def kernel(x, w1, w2, W1, b1, W2, rand_u, steps, **kw):
    from concourse.bass_utils import run_bass_kernel_spmd

    assert int(steps) == STEPS
    x = np.asarray(x, np.float32)
    in_maps = _make_in_maps(
        x,
        np.asarray(w1, np.float32),
        np.asarray(w2, np.float32),
        np.asarray(W1, np.float32),
        np.asarray(b1, np.float32),
        np.asarray(W2, np.float32),
        np.asarray(rand_u, np.float32),
    )
    nc = _get_module()
    res = run_bass_kernel_spmd(nc, in_maps, core_ids=list(range(B)))
    _CACHE["last_results"] = res
    out = np.empty((B, H, WID, C), np.float32)
    for b in range(B):
        out[b] = _unpack_x(res.results[b]["xout"])
    return out


# revision 15
# speedup vs baseline: 1.1236x; 1.0824x over previous
"""Trainium2 Bass kernel for nn_BasicNCAModel — fp8 DoubleRow mm1 variant.

Same structure as the f16 kernel (unified stack, carried mm2, fire trick)
with mm1 in fp8e4: the dy=-1 and dy=+1 taps fuse into ONE DoubleRow matmul
(virtual K=256, 2 multiplies/cycle), the center tap is a normal fp8 matmul,
so mm1 is 2 matmuls per (j,g) instead of 3. The stack is fp8 pitch-128
(DoubleRow needs a single-stride moving AP: k-tile pair stride 256 B), fed
from a pitch-130 fp8 shadow x8 whose wrap cols give the dx=+-1 shifts their
circular reads. Weights are scaled by SA=32 to sit in fp8e4's normal range
(fire weight 240 = max normal; effective M = 240/32 = 7.5 >> |h+b1|), and
the 1/SA is folded into W2 (f16), so no extra scaling ops anywhere.
"""

import sys

if "/opt/trn_rl_repo" not in sys.path:
    sys.path.insert(0, "/opt/trn_rl_repo")

import numpy as np
import ml_dtypes

F8 = ml_dtypes.float8_e4m3

C = 24
NIC = 4
H = 128
WID = 128
HID = 128
STEPS = 8
B = 8
G = 4
RG = 32
PITCH = 132    # f16 master pitch
GROWS = 34
FB = GROWS * PITCH
SP8 = 128      # fp8 stack pitch (contiguous pixel rows)
SROWS = 130
SFL8 = SROWS * SP8
X8P = 130      # fp8 shadow pitch (wrap col + 128 + wrap col)
X8L = 32 * X8P
TW = 512
SA = 32.0      # fp8 weight scale
MF8 = 240.0    # fire weight (fp8e4 max normal); effective M = MF8/SA
SC_FD = 352

_CACHE = {}


def _build_module():
    from concourse import bacc, mybir, tile
    from concourse.ap import AP

    f32 = mybir.dt.float32
    f16 = mybir.dt.float16
    f8 = mybir.dt.float8e4
    Alu = mybir.AluOpType
    Act = mybir.ActivationFunctionType
    DR = mybir.MatmulPerfMode.DoubleRow

    nc = bacc.Bacc(
        "TRN2",
        target_bir_lowering=False,
        debug=False,
        enable_asserts=False,
        num_devices=8,
    )

    apdr = nc.dram_tensor("apdr", [128, 256], f8, kind="ExternalInput").ap()
    apc = nc.dram_tensor("apc", [128, 128], f8, kind="ExternalInput").ap()
    w2p = nc.dram_tensor("w2p", [128, 32], f16, kind="ExternalInput").ap()
    b1col = nc.dram_tensor("b1col", [128, 1], f32, kind="ExternalInput").ap()
    x16in = nc.dram_tensor("x16in", [128, FB], f16, kind="ExternalInput").ap()
    x8in = nc.dram_tensor("x8in", [128, X8L], f8, kind="ExternalInput").ap()
    firein = nc.dram_tensor("firein", [128, 4096], f8, kind="ExternalInput").ap()
    stkin = nc.dram_tensor("stkin", [128, SFL8], f8, kind="ExternalInput").ap()
    xout = nc.dram_tensor("xout", [128, 4096], f32, kind="ExternalOutput").ap()

    with tile.TileContext(nc) as tc:
        import contextlib

        with contextlib.ExitStack() as ctx:
            sing = ctx.enter_context(tc.tile_pool(name="sing", bufs=1))
            hpool = ctx.enter_context(tc.tile_pool(name="h", bufs=6, space="PSUM"))
            dxpool = ctx.enter_context(tc.tile_pool(name="dx", bufs=2, space="PSUM"))
            hsb = ctx.enter_context(tc.tile_pool(name="hsb", bufs=8))

            x16a = sing.tile([128, FB], f16)
            x16b = sing.tile([128, FB], f16)
            x8 = sing.tile([128, X8L], f8)
            xof = sing.tile([128, 4096], f32)
            fire = sing.tile([128, 4096], f8)
            A8 = sing.tile([128, 256], f8)
            AC = sing.tile([128, 128], f8)
            W2s = sing.tile([128, 32], f16)
            zeros = sing.tile([128, TW - SC_FD], f32)
            b1c = sing.tile([128, 1], f32)
            stk = [sing.tile([128, SFL8], f8, name=f"stk_{b}") for b in range(2)]

            nc.scalar.dma_start(A8[:], apdr[:])
            nc.scalar.dma_start(AC[:], apc[:])
            QS = [nc.sync, nc.gpsimd, nc.sync, nc.scalar]
            for g in range(G):
                lo, hi = (32 * g + 4) * SP8, (32 * g + 22) * SP8
                QS[g].dma_start(stk[0][:, lo:hi], stkin[:, lo:hi])
            gaps = [(0, 4), (22, 36), (54, 68), (86, 100), (118, 130)]
            for k, (a, b) in enumerate(gaps):
                QS[k % 3].dma_start(
                    stk[0][:, a * SP8 : b * SP8],
                    stkin[:, a * SP8 : b * SP8],
                )
            nc.gpsimd.dma_start(W2s[:], w2p[:])
            nc.gpsimd.dma_start(b1c[:], b1col[:])
            nc.sync.dma_start(x16a[:], x16in[:])
            nc.sync.dma_start(x8[:], x8in[:])
            nc.sync.dma_start(fire[:], firein[:])
            nc.gpsimd.memset(stk[1][64:128, :], 0.0)
            nc.gpsimd.memset(zeros[:], 0.0)

            xf16 = [x16a, x16b]
            fire3 = fire[:].rearrange("p (r w) -> p r w", w=128)
            xo3 = xof[:].rearrange("p (r w) -> p r w", w=128)
            x8v = x8[:].rearrange("p (r w) -> p r w", w=X8P)
            a8v = A8[:].rearrange("p (t n) -> p t n", t=2)
            SLICES = [(0, 4), (4, 16), (16, 24), (24, 32)]
            JORD = [2, 3, 1, 4, 5, 0, 6, 7]
            TRIGGER = {0: 0, 1: 1, 5: 2, 7: 3}

            def emit_slice(s, sl):
                sb = stk[(s + 1) % 2]
                la, lb = SLICES[sl]
                for g in range(G):
                    dlo = (32 * g + 1 + la) * SP8
                    dhi = (32 * g + 1 + lb) * SP8
                    for d in range(3):
                        eng = [nc.sync, nc.gpsimd][(g + d) % 2]
                        eng.dma_start(
                            sb[24 * d : 24 * d + 24, dlo:dhi],
                            x8v[32 * g : 32 * g + 24, la:lb, d : d + 128],
                        )

            def emit_fire(s):
                sb = (s + 1) % 2
                for g in range(G):
                    nc.gpsimd.dma_start(
                        stk[sb][72:73, (32 * g + 1) * SP8 : (32 * g + 33) * SP8],
                        fire3[32 * g + s + 1 : 32 * g + s + 2, :, :],
                    )

            state = {"prev": None}

            def mm2_update(s, j, hss, last):
                r0 = 4 * j + 1
                xc = xf16[s % 2][:].rearrange("p (r w) -> p r w", w=PITCH)
                xn = xf16[(s + 1) % 2][:].rearrange("p (r w) -> p r w", w=PITCH)
                dxt = dxpool.tile([128, TW], f32, tag="dx", name=f"dx_{s}_{j}")
                for g in range(G):
                    nc.tensor.matmul(
                        dxt[32 * g : 32 * g + 32, :],
                        W2s[:],
                        hss[g][:],
                        start=True,
                        stop=True,
                        tile_position=(0, 32 * g),
                    )
                dx3 = dxt[:].rearrange("p (a b) -> p a b", b=128)
                if last:
                    nc.vector.tensor_tensor(
                        xo3[:, 4 * j : 4 * j + 4, :],
                        dx3,
                        xc[:, r0 : r0 + 4, 1:129],
                        Alu.add,
                    )
                    return
                nc.vector.tensor_tensor(
                    xn[:, r0 : r0 + 4, 1:129],
                    dx3,
                    xc[:, r0 : r0 + 4, 1:129],
                    Alu.add,
                )
                nc.vector.tensor_copy(
                    xn[:, r0 : r0 + 4, 0:1], xn[:, r0 : r0 + 4, 128:129]
                )
                nc.vector.tensor_copy(
                    xn[:, r0 : r0 + 4, 129:130], xn[:, r0 : r0 + 4, 1:2]
                )
                # fp8 shadow of the updated rows (feeds next stack build)
                nc.vector.tensor_copy(
                    x8v[:, 4 * j : 4 * j + 4, 0:130],
                    xn[:, r0 : r0 + 4, 0:130],
                )
                if j in TRIGGER:
                    emit_slice(s, TRIGGER[j])
                    nb = stk[(s + 1) % 2]
                    if TRIGGER[j] == 0:
                        # halo srow 129 (image row 0) <- srow 1
                        nc.gpsimd.tensor_copy(
                            nb[:80, 129 * SP8 : 130 * SP8],
                            nb[:80, 1 * SP8 : 2 * SP8],
                        )
                    if TRIGGER[j] == 3:
                        # halo srow 0 (image row 127) <- srow 128
                        nc.gpsimd.tensor_copy(
                            nb[:80, 0:SP8],
                            nb[:80, 128 * SP8 : 129 * SP8],
                        )

            for s in range(STEPS):
                last = s + 1 == STEPS
                if not last:
                    emit_fire(s)
                sT = stk[s % 2][:]
                for j in JORD:
                    hss = []
                    for g in range(G):
                        sr0 = 32 * g + 4 * j + 1
                        ht = hpool.tile(
                            [128, TW], f32, tag="h", name=f"h_{s}_{j}_{g}"
                        )
                        # DoubleRow: k-tile 0 = dy=-1 rows, k-tile 1 =
                        # dy=+1 rows (pair stride 2 srows = 256 B)
                        rhs_dr = AP(
                            tensor=sT.tensor,
                            offset=sT.offset + (sr0 - 1) * SP8,
                            ap=[[SFL8, 128], [2 * SP8, 2], [1, 512]],
                        )
                        nc.tensor.matmul(
                            ht[:, :],
                            a8v,
                            rhs_dr,
                            start=True,
                            stop=False,
                            perf_mode=DR,
                            tile_position=(0, 0),
                        )
                        rhs_c = sT[:, sr0 * SP8 : sr0 * SP8 + TW]
                        nc.tensor.matmul(
                            ht[:, :],
                            AC[:],
                            rhs_c,
                            start=False,
                            stop=True,
                            tile_position=(0, 0),
                        )
                        hs = hsb.tile(
                            [128, TW], f16, tag="hsb", name=f"hs_{s}_{j}_{g}"
                        )
                        nc.scalar.activation(
                            hs[:, :SC_FD],
                            ht[:, :SC_FD],
                            Act.Relu,
                            bias=b1c[:],
                        )
                        nc.vector.scalar_tensor_tensor(
                            hs[:, SC_FD:],
                            ht[:, SC_FD:],
                            b1c[:],
                            zeros[:],
                            Alu.add,
                            Alu.max,
                        )
                        hss.append(hs)
                    if state["prev"] is not None:
                        mm2_update(*state["prev"])
                    state["prev"] = (s, j, hss, last)
            mm2_update(*state["prev"])

            nc.sync.dma_start(xout[:, :], xof[:, :])

    nc.compile()
    return nc


def _get_module():
    if "nc" not in _CACHE:
        _CACHE["nc"] = _build_module()
    return _CACHE["nc"]


def _prep_weights(w1, w2, W1, b1, W2):
    A = np.zeros((9, HID, C), np.float32)
    for t in range(9):
        dy, dxx = t // 3 - 1, t % 3 - 1
        A[t] = (
            W1[:, 24:48] * w1[dy + 1, dxx + 1, 0][None, :]
            + W1[:, 48:72] * w2[dy + 1, dxx + 1, 0][None, :]
        )
    A[4] += W1[:, :24]
    apdr = np.zeros((128, 256), np.float32)
    apc = np.zeros((128, 128), np.float32)
    for d in range(3):
        rows = slice(24 * d, 24 * d + 24)
        apdr[rows, 0:128] = SA * A[0 * 3 + d].T      # dy = -1
        apdr[rows, 128:256] = SA * A[2 * 3 + d].T    # dy = +1
        apc[rows, :] = SA * A[1 * 3 + d].T           # dy = 0
    apc[72, :] = MF8
    w2pk = np.zeros((128, 32), np.float32)
    w2pk[:, NIC:C] = W2[NIC:C].T / SA
    b1c = (SA * b1 - MF8).reshape(128, 1).astype(np.float32)
    return apdr.astype(F8), apc.astype(F8), w2pk.astype(np.float16), b1c


def _pack_x(ximg):
    """[128,128,24] image -> [128, FB] haloed channel-major fp16."""
    xin = np.zeros((128, FB), np.float32)
    cols = (np.arange(-1, 129)) % WID
    for g in range(G):
        rows = (np.arange(-1, 33) + 32 * g) % H
        blk = ximg[rows][:, cols, :]
        buf = np.zeros((24, GROWS, PITCH), np.float32)
        buf[:, :, :130] = np.transpose(blk, (2, 0, 1))
        xin[32 * g : 32 * g + 24] = buf.reshape(24, FB)
    return xin.astype(np.float16)


def _pack_x8(ximg):
    """[128,128,24] image -> [128, X8L] fp8 shadow (pitch 130, wrap cols).
    Matches the device fp16->fp8 rounding closely enough (direct f32->fp8)."""
    x8 = np.zeros((128, RG, X8P), np.float32)
    cols = (np.arange(-1, 129)) % WID
    for g in range(G):
        rows = np.arange(0, 32) + 32 * g
        blk = ximg[rows][:, cols, :]  # [32, 130, 24]
        x8[32 * g : 32 * g + 24] = np.transpose(blk, (2, 0, 1))
    return x8.reshape(128, X8L).astype(np.float16).astype(F8)


def _unpack_x(xo):
    img = np.empty((H, WID, C), np.float32)
    for g in range(G):
        blk = xo[32 * g : 32 * g + 24].reshape(24, RG, WID)
        img[32 * g : 32 * g + 32] = np.transpose(blk, (1, 2, 0))
    return img


def _build_stack0(ximg, fire0):
    """Host: step-0 unified fp8 stack [128, SFL8] (srow i = image row i-1)."""
    stk0 = np.zeros((128, SROWS, SP8), np.float32)
    rows = np.arange(-1, 129) % H
    for d in range(3):
        cols = (np.arange(0, 128) + (d - 1)) % WID
        blk = ximg[rows][:, cols, :]  # [130, 128, 24]
        stk0[24 * d : 24 * d + 24] = np.transpose(blk, (2, 0, 1))
    stk0[72] = fire0[rows]
    return (
        stk0.reshape(128, SFL8).astype(np.float16).astype(F8)
    )


def _make_in_maps(x, w1, w2, W1, b1, W2, rand_u):
    apdr, apc, w2pk, b1c = _prep_weights(w1, w2, W1, b1, W2)
    in_maps = []
    for b in range(B):
        u = rand_u[:, b, :, :, 0].reshape(STEPS, H * WID)
        fire8 = np.zeros((128, 4096), F8)
        for g in range(G):
            for s in range(STEPS):
                fire8[32 * g + s] = (
                    u[s, g * 4096 : (g + 1) * 4096] < 0.5
                ).astype(F8)
        ximg = np.asarray(x[b], np.float32)
        fire0 = (u[0].reshape(H, WID) < 0.5).astype(np.float32)
        in_maps.append(
            {
                "apdr": apdr,
                "apc": apc,
                "w2p": w2pk,
                "b1col": b1c,
                "x16in": _pack_x(ximg),
                "x8in": _pack_x8(ximg),
                "firein": fire8,
                "stkin": _build_stack0(ximg, fire0),
            }
        )
    return in_maps


def kernel(x, w1, w2, W1, b1, W2, rand_u, steps, **kw):
    from concourse.bass_utils import run_bass_kernel_spmd

    assert int(steps) == STEPS
    x = np.asarray(x, np.float32)
    in_maps = _make_in_maps(
        x,
        np.asarray(w1, np.float32),
        np.asarray(w2, np.float32),
        np.asarray(W1, np.float32),
        np.asarray(b1, np.float32),
        np.asarray(W2, np.float32),
        np.asarray(rand_u, np.float32),
    )
    nc = _get_module()
    res = run_bass_kernel_spmd(nc, in_maps, core_ids=list(range(B)))
    _CACHE["last_results"] = res
    out = np.empty((B, H, WID, C), np.float32)
    for b in range(B):
        out[b] = _unpack_x(res.results[b]["xout"])
    return out
